# revision 10
# baseline (speedup 1.0000x reference)
"""ConfidenceAttention Trainium2 kernel (8 NeuronCores, batch-parallel).

Reference math (per batch b, n == 256 keys/queries, 8 heads x 64):
  q = query @ Wq; k = key @ Wk; v = value @ Wv            (biases are zero)
  conf[j] = sigmoid(rel_conf[b,j] @ w_rel) * sigmoid(abs_conf[j] @ w_abs)
  s[h,q,j] = (q_h . k_h[j]) / 8 * conf[j]
  p = softmax_j(s);  out = (p @ v) @ Wo
  out_conf[b,q,:] = sum_j p[...,j] * abs_conf[b] = abs_conf[b]  (sum_j p = 1)

Sharding: batch dim (256) split over 8 cores, 32 batches/core.
abs_confidence + all weights replicated; no collectives needed.
out_conf computed on host (exact broadcast identity).

Engine layout per batch: PE does transposes + all matmuls (bf16 fronted,
fp32r for score/output projections); ACT does exp(+row-sum accumulation)
and some PSUM evacuation; DVE the rest of evacuation; GPSIMD the softmax
normalize and cast-DMAs.
"""
import numpy as np
from contextlib import ExitStack

import concourse.bass as bass
import concourse.mybir as mybir
import concourse.tile as tile
from concourse import bacc
from concourse.bass_utils import run_bass_kernel_spmd
from concourse.masks import make_identity

B, N, HID, CD, NH, HD = 256, 256, 512, 256, 8, 64
NCORES = 8
f32 = mybir.dt.float32
f32r = mybir.dt.float32r
bf16 = mybir.dt.bfloat16
AF = mybir.ActivationFunctionType

_cache = {}


def build_nc(bpc: int):
    """Build the per-core program processing `bpc` batches."""
    nc = bacc.Bacc()

    q_d = nc.declare_dram_parameter("query", [bpc, N, HID], f32, isOutput=False)
    k_d = nc.declare_dram_parameter("key", [bpc, N, HID], f32, isOutput=False)
    v_d = nc.declare_dram_parameter("value", [bpc, N, HID], f32, isOutput=False)
    rc_d = nc.declare_dram_parameter("rel_confidence", [bpc, N, CD], f32, isOutput=False)
    ac_d = nc.declare_dram_parameter("abs_confidence", [B, 1, CD], f32, isOutput=False)
    wq_d = nc.declare_dram_parameter("Wq", [HID, HID], f32, isOutput=False)
    wk_d = nc.declare_dram_parameter("Wk", [HID, HID], f32, isOutput=False)
    wv_d = nc.declare_dram_parameter("Wv", [HID, HID], f32, isOutput=False)
    wo_d = nc.declare_dram_parameter("Wo", [HID, HID], f32, isOutput=False)
    wr_d = nc.declare_dram_parameter("w_rel", [CD, 1], f32, isOutput=False)
    wa_d = nc.declare_dram_parameter("w_abs", [CD, 1], f32, isOutput=False)
    out_d = nc.declare_dram_parameter("out", [bpc, N, HID], f32, isOutput=True)

    with ExitStack() as ctx:
        tc = ctx.enter_context(tile.TileContext(nc))
        const = ctx.enter_context(tc.tile_pool(name="const", bufs=1))
        sb = ctx.enter_context(tc.tile_pool(name="sb", bufs=1))
        ps_mm = ctx.enter_context(tc.tile_pool(name="ps_mm", bufs=1, space="PSUM"))
        ps_s = ctx.enter_context(tc.tile_pool(name="ps_s", bufs=1, space="PSUM"))
        ps_tp = ctx.enter_context(tc.tile_pool(name="ps_tp", bufs=1, space="PSUM"))
        ps_o = ctx.enter_context(tc.tile_pool(name="ps_o", bufs=1, space="PSUM"))

        # ---- constants ----
        ident = const.tile([128, 128], bf16)
        make_identity(nc, ident)
        identf = const.tile([128, 128], f32)
        make_identity(nc, identf)

        wq_sb = const.tile([128, 4, HID], f32r)
        nc.gpsimd.dma_start(out=wq_sb[:], in_=wq_d.ap().rearrange("(c p) d -> p c d", p=128))
        wk_sb = const.tile([128, 4, HID], f32r)
        nc.gpsimd.dma_start(out=wk_sb[:], in_=wk_d.ap().rearrange("(c p) d -> p c d", p=128))
        wv_sb = const.tile([128, 4, HID], f32r)
        nc.gpsimd.dma_start(out=wv_sb[:], in_=wv_d.ap().rearrange("(c p) d -> p c d", p=128))
        wo_sb = const.tile([128, 4, HID], f32r)
        nc.gpsimd.dma_start(out=wo_sb[:], in_=wo_d.ap().rearrange("(c p) d -> p c d", p=128))
        wr_sb = const.tile([128, 2], f32r)
        nc.gpsimd.dma_start(out=wr_sb[:], in_=wr_d.ap().rearrange("(c p) one -> p (c one)", p=128))
        wa_sb = const.tile([128, 2], bf16)
        nc.gpsimd.dma_start(out=wa_sb[:], in_=wa_d.ap().rearrange("(c p) one -> p (c one)", p=128))

        ones_f = const.tile([1, 128], f32)
        nc.vector.memset(ones_f[:], 1.0)
        ones_col = const.tile([1, 128], f32r)
        nc.vector.tensor_copy(ones_col[:], ones_f[:])

        # ---- abs_c = sigmoid(abs_confidence @ w_abs)  -> [1, B] along free
        aa = const.tile([128, 2, CD], bf16)
        nc.gpsimd.dma_start(out=aa[:], in_=ac_d.ap().rearrange("(t p) one c -> p t (one c)", p=128))
        at = const.tile([128, 2, B], bf16)
        for c in range(2):
            tp = ps_tp.tile([128, 256], bf16, tag="tp", bufs=2)
            for t in range(2):
                nc.tensor.transpose(tp[:, t * 128:(t + 1) * 128],
                                    aa[:, t, c * 128:(c + 1) * 128], ident[:])
            nc.vector.tensor_copy(at[:, c, :], tp[:])
        la = ps_o.tile([1, B], f32, tag="o", bufs=2)
        for c in range(2):
            nc.tensor.matmul(la[:], wa_sb[:, c:c + 1], at[:, c, :],
                             start=(c == 0), stop=(c == 1))
        abs_e = const.tile([1, B], f32)
        nc.scalar.activation(abs_e[:], la[:], AF.Exp, bias=0.0, scale=-1.0)
        abs_c = const.tile([1, B], f32)
        nc.vector.tensor_scalar_add(abs_c[:], abs_e[:], 1.0)
        nc.vector.reciprocal(abs_c[:], abs_c[:])

        # ---- per-batch loop ----
        for b in range(bpc):
            # [1] input DMAs (f32 -> bf16 cast via SWDGE)
            xq = sb.tile([128, 2, HID], f32, tag="xq", bufs=3)
            nc.sync.dma_start(out=xq[:], in_=q_d.ap()[b].rearrange("(t p) d -> p t d", p=128))
            xk = sb.tile([128, 2, HID], f32, tag="xk", bufs=3)
            nc.sync.dma_start(out=xk[:], in_=k_d.ap()[b].rearrange("(t p) d -> p t d", p=128))
            xv = sb.tile([128, 2, HID], f32, tag="xv", bufs=3)
            nc.sync.dma_start(out=xv[:], in_=v_d.ap()[b].rearrange("(t p) d -> p t d", p=128))
            rr = sb.tile([128, 2, CD], f32, tag="rr", bufs=3)
            nc.sync.dma_start(out=rr[:], in_=rc_d.ap()[b].rearrange("(t p) c -> p t c", p=128))

            # [2] transposes: x^T [hid, n] as [128, 4(c), 256]
            xqt = sb.tile([128, 4, N], f32r, tag="xqt", bufs=2)
            xkt = sb.tile([128, 4, N], f32r, tag="xkt", bufs=2)
            xvt = sb.tile([128, 4, N], f32r, tag="xvt", bufs=2)
            for src, dst in ((xq, xqt), (xk, xkt), (xv, xvt)):
                for c in range(4):
                    tp = ps_tp.tile([128, 256], f32, tag="tp", bufs=2)
                    for t in range(2):
                        nc.tensor.transpose(tp[:, t * 128:(t + 1) * 128],
                                            src[:, t, c * 128:(c + 1) * 128], identf[:])
                    nc.vector.tensor_copy(dst[:, c, :], tp[:])
            rt = sb.tile([128, 2, N], f32r, tag="rt", bufs=2)
            for c in range(2):
                tp = ps_tp.tile([128, 256], f32, tag="tp", bufs=2)
                for t in range(2):
                    nc.tensor.transpose(tp[:, t * 128:(t + 1) * 128],
                                        rr[:, t, c * 128:(c + 1) * 128], identf[:])
                nc.vector.tensor_copy(rt[:, c, :], tp[:])

            # [3] conf row: sigmoid(R @ w_rel) * abs_c   [1, 256] (f32r)
            lrel = ps_o.tile([1, N], f32, tag="o", bufs=2)
            for c in range(2):
                nc.tensor.matmul(lrel[:], wr_sb[:, c:c + 1], rt[:, c, :],
                                 start=(c == 0), stop=(c == 1))
            relc = sb.tile([1, N], f32, tag="relc", bufs=2)
            nc.scalar.activation(relc[:], lrel[:], AF.Exp, bias=0.0, scale=-1.0)
            nc.vector.tensor_scalar_add(relc[:], relc[:], 1.0)
            nc.vector.reciprocal(relc[:], relc[:])
            conf = sb.tile([1, N], f32r, tag="conf", bufs=2)
            nc.vector.tensor_tensor(conf[:], relc[:], abs_c[:], mybir.AluOpType.mult)
            # broadcast conf over 128 partitions
            cbp = ps_s.tile([128, N], f32, tag="s", bufs=2)
            nc.tensor.matmul(cbp[:], ones_col[:], conf[:], start=True, stop=True)
            cb = sb.tile([128, N], f32, tag="cb", bufs=2)
            nc.vector.tensor_copy(cb[:], cbp[:])

            # [4] projections
            # q^T, k^T: [hid_out, n] as [128, 4(m), 256]; q scaled by 1/8
            qt = sb.tile([128, 4, N], f32r, tag="qt", bufs=2)
            kt = sb.tile([128, 4, N], f32r, tag="kt", bufs=2)
            for m in range(4):
                qp = ps_mm.tile([128, N], f32, tag="mm", bufs=2)
                for c in range(4):
                    nc.tensor.matmul(qp[:], wq_sb[:, c, m * 128:(m + 1) * 128],
                                     xqt[:, c, :], start=(c == 0), stop=(c == 3))
                nc.scalar.activation(qt[:, m, :], qp[:], AF.Copy, bias=0.0, scale=0.125)
                kp = ps_mm.tile([128, N], f32, tag="mm", bufs=2)
                for c in range(4):
                    nc.tensor.matmul(kp[:], wk_sb[:, c, m * 128:(m + 1) * 128],
                                     xkt[:, c, :], start=(c == 0), stop=(c == 3))
                # fold conf[j] into k^T columns during evacuation
                nc.vector.tensor_tensor(kt[:, m, :], kp[:], cb[:], mybir.AluOpType.mult)
            # v natural [n, hid] as [128, 2(t), 512] bf16
            vn = sb.tile([128, 2, HID], bf16, tag="vn", bufs=2)
            for t in range(2):
                vp = ps_mm.tile([128, HID], f32, tag="mm", bufs=2)
                for c in range(4):
                    nc.tensor.matmul(vp[:], xvt[:, c, t * 128:(t + 1) * 128],
                                     wv_sb[:, c, :], start=(c == 0), stop=(c == 3))
                nc.vector.tensor_copy(vn[:, t, :], vp[:])

            # [5] scores + exp + row sums;  p: [128, 16(h,qc), 256] bf16
            p_sb = sb.tile([128, 16, N], bf16, tag="p", bufs=3)
            l_all = sb.tile([128, 16], f32, tag="l", bufs=3)
            for h in range(NH):
                po = 64 * (h % 2)
                m = h // 2
                for qc in range(2):
                    sp = ps_s.tile([128, N], f32, tag="s", bufs=2)
                    nc.tensor.matmul(sp[:],
                                     qt[po:po + 64, m, qc * 128:(qc + 1) * 128],
                                     kt[po:po + 64, m, :], start=True, stop=True)
                    idx = h * 2 + qc
                    nc.scalar.activation(p_sb[:, idx, :], sp[:], AF.Exp,
                                         bias=0.0, scale=1.0,
                                         accum_out=l_all[:, idx:idx + 1])

            # [6] normalize p (gpsimd), r = 1/l
            r_all = sb.tile([128, 16], f32, tag="r", bufs=3)
            nc.vector.reciprocal(r_all[:], l_all[:])
            for idx in range(16):
                nc.vector.tensor_scalar_mul(p_sb[:, idx, :], p_sb[:, idx, :],
                                            r_all[:, idx:idx + 1])

            # [7] transpose p~ -> p~^T [k, q]: [128, 16(h,kc), 256] bf16
            ptT = sb.tile([128, 16, N], bf16, tag="ptT", bufs=3)
            for h in range(NH):
                for kc in range(2):
                    pp = ps_tp.tile([128, 256], bf16, tag="tp", bufs=2)
                    for qc in range(2):
                        nc.tensor.transpose(pp[:, qc * 128:(qc + 1) * 128],
                                            p_sb[:, h * 2 + qc, kc * 128:(kc + 1) * 128],
                                            ident[:])
                    nc.vector.tensor_copy(ptT[:, h * 2 + kc, :], pp[:])

            # [8] attention: o^T[hd, q] per head -> oT [128, 4(m), 256] f32r
            oT = sb.tile([128, 4, N], f32r, tag="oT", bufs=2)
            for h in range(NH):
                op = ps_o.tile([64, N], f32, tag="o", bufs=2)
                for kc in range(2):
                    nc.tensor.matmul(op[:], vn[:, kc, h * 64:(h + 1) * 64],
                                     ptT[:, h * 2 + kc, :],
                                     start=(kc == 0), stop=(kc == 1))
                nc.scalar.copy(oT[64 * (h % 2):64 * (h % 2) + 64, h // 2, :], op[:])

            # [9] final projection -> [n, hid] f32, DMA out
            fin = sb.tile([128, 2, HID], f32, tag="fin", bufs=3)
            for t in range(2):
                fp = ps_mm.tile([128, HID], f32, tag="mm", bufs=2)
                for c in range(4):
                    nc.tensor.matmul(fp[:], oT[:, c, t * 128:(t + 1) * 128],
                                     wo_sb[:, c, :], start=(c == 0), stop=(c == 3))
                nc.vector.tensor_copy(fin[:, t, :], fp[:])
            nc.sync.dma_start(out=out_d.ap()[b].rearrange("(t p) d -> p t d", p=128),
                              in_=fin[:])

    nc.compile()
    return nc


def get_nc(bpc: int):
    if bpc not in _cache:
        _cache[bpc] = build_nc(bpc)
    return _cache[bpc]


def make_in_maps(inputs, bpc: int):
    per = B // NCORES
    in_maps = []
    for c in range(NCORES):
        lo = c * per
        m = {
            "query": np.ascontiguousarray(inputs["query"][lo:lo + bpc]),
            "key": np.ascontiguousarray(inputs["key"][lo:lo + bpc]),
            "value": np.ascontiguousarray(inputs["value"][lo:lo + bpc]),
            "rel_confidence": np.ascontiguousarray(inputs["rel_confidence"][lo:lo + bpc]),
            "abs_confidence": np.ascontiguousarray(inputs["abs_confidence"]),
            "Wq": np.ascontiguousarray(inputs["Wq"]),
            "Wk": np.ascontiguousarray(inputs["Wk"]),
            "Wv": np.ascontiguousarray(inputs["Wv"]),
            "Wo": np.ascontiguousarray(inputs["Wo"]),
            "w_rel": np.ascontiguousarray(inputs["w_rel"]),
            "w_abs": np.ascontiguousarray(inputs["w_abs"]),
        }
        in_maps.append(m)
    return in_maps


def run_device(inputs, bpc: int = B // NCORES):
    """Run the device kernel; returns `out` rows [c*32 : c*32+bpc] per core."""
    nc = get_nc(bpc)
    in_maps = make_in_maps(inputs, bpc)
    res = run_bass_kernel_spmd(nc, in_maps, list(range(NCORES)))
    outs = [res.results[c]["out"] for c in range(NCORES)]
    return outs


def kernel(**inputs) -> tuple[np.ndarray, np.ndarray]:
    inputs = {k: np.asarray(v, np.float32) if np.asarray(v).dtype != np.float32
              else np.asarray(v) for k, v in inputs.items()}
    bpc = B // NCORES
    outs = run_device(inputs, bpc)
    out = np.concatenate(outs, axis=0)
    # out_conf == broadcast(abs_confidence): softmax rows sum to 1 and the
    # second einsum's value tensor is constant across keys.
    out_conf = np.broadcast_to(
        np.asarray(inputs["abs_confidence"], np.float32), (B, N, CD)
    ).copy()
    return out, out_conf
